# revision 6
# baseline (speedup 1.0000x reference)
"""ClashLoss kernel for Trainium2 (8 NeuronCores, batch-parallel).

Math: for each batch b, count pairs (n, m), n != m, with
    dist(n, m) < radii[n] + radii[m]   and   dist(n, m) > EPS.
Using s_n = |c_n|^2 - r_n^2, the clash condition dist^2 < (r_n + r_m)^2 is
    G[n, m] = dot(c_n, c_m) + r_n r_m - s_n/2 - s_m/2 > 0,
i.e. a 6-dim dot product u_n . v_m with
    u_n = (x, y, z, r_n, -s_n/2, 1)     (matmul stationary side)
    v_m = (x, y, z, r_m, 1, -s_m/2)     (matmul moving side)
The PE computes G tile-by-tile into PSUM; ACT (Sign + accumulate) and DVE
(tensor_scalar is_gt + accumulate) count positives per 512x512 super-block.
Symmetry: only upper-triangular super-blocks are computed (weight 2), the
diagonal super-blocks once (weight 1).  The diagonal n == m has
G[n,n] = 2 r_n^2 > 0 always, so exactly N diagonal hits are subtracted on
the host.

Raw-bass implementation (not Tile): fp32 matmuls only support a single
sync-wait in codegen, so semaphores are placed by hand -- at most one wait
per matmul, standalone wait instructions elsewhere.
"""

import numpy as np

N = 4096
B = 8
K = 6  # augmented dot-product length
SUPER = 512  # super-block edge (4 PSUM banks wide)
NSUP = N // SUPER  # 8
ROWT = 128  # rows per matmul (output partitions)
CHUNK_FD = SUPER * 4  # 2048 f32 = 4 PSUM banks
EPS = 1e-8

# super-block schedule: (R, C, weight); R <= C
SCHEDULE = [(r, c, 1 if r == c else 2) for r in range(NSUP) for c in range(r, NSUP)]
NSLOTS = len(SCHEDULE)  # 36

# engine assignment: ACT is a bit faster per chunk; give it more supers.
ENGINE = ["act" if (i % 9) < 5 else "dve" for i in range(NSLOTS)]
# compact per-engine slot index for each super
SLOT_IDX = []
_na = _nd = 0
for _i in range(NSLOTS):
    if ENGINE[_i] == "act":
        SLOT_IDX.append(_na)
        _na += 1
    else:
        SLOT_IDX.append(_nd)
        _nd += 1
N_ACT, N_DVE = _na, _nd

_CACHE = {}


def _build():
    """Build the raw-bass SPMD program (same program for all cores)."""
    import concourse.bass as bass
    from concourse import mybir

    nc = bass.Bass("TRN2", target_bir_lowering=False, debug=False)
    f32 = mybir.dt.float32

    u_dram = nc.dram_tensor("u6", [K, N], f32, kind="ExternalInput").ap()
    v_dram = nc.dram_tensor("v6", [K, N], f32, kind="ExternalInput").ap()
    out_dram = nc.dram_tensor(
        "counts", [128, N_ACT + N_DVE], f32, kind="ExternalOutput"
    ).ap()

    # consumer bookkeeping: for super i, which engine consumes it and the
    # cumulative per-engine consumption count up to and including i.
    cons_count = []  # (engine, count_after_i)
    na = nd = 0
    for i in range(NSLOTS):
        if ENGINE[i] == "act":
            na += 1
            cons_count.append(("act", na))
        else:
            nd += 1
            cons_count.append(("dve", nd))
    n_act_total, n_dve_total = na, nd

    with (
        nc.sbuf_tensor([K, N], f32) as u_sb,
        nc.sbuf_tensor([K, N], f32) as v_sb,
        nc.sbuf_tensor([128, max(1, N_ACT)], f32) as act_slots,
        nc.sbuf_tensor([128, max(1, N_DVE)], f32) as dve_slots,
        nc.sbuf_tensor([128, NSLOTS], f32) as act_dummy,
        nc.sbuf_tensor([128, NSLOTS], f32) as dve_dummy,
        nc.psum_tensor([128, CHUNK_FD], f32) as chunk0,
        nc.psum_tensor([128, CHUNK_FD], f32) as chunk1,
        nc.semaphore("DMA_IN") as s_in,
        nc.semaphore("PROD") as s_prod,
        nc.semaphore("CACT") as s_cact,
        nc.semaphore("CDVE") as s_cdve,
        nc.semaphore("DMA_OUT") as s_out,
        nc.Block() as block,
    ):
        chunks = [chunk0, chunk1]

        @block.sync
        def _(sync):
            sync.dma_start(out=u_sb[:, :], in_=u_dram).then_inc(s_in, 16)
            sync.dma_start(out=v_sb[:, :], in_=v_dram).then_inc(s_in, 16)
            sync.wait_ge(s_cact, n_act_total)
            sync.wait_ge(s_cdve, n_dve_total)
            sync.dma_start(
                out=out_dram[:, 0:N_ACT], in_=act_slots[:, :]
            ).then_inc(s_out, 16)
            sync.dma_start(
                out=out_dram[:, N_ACT : N_ACT + N_DVE], in_=dve_slots[:, :]
            ).then_inc(s_out, 16)
            sync.wait_ge(s_out, 32)

        @block.tensor
        def _(tensor):
            for i, (R, C, _w) in enumerate(SCHEDULE):
                chunk = chunks[i % 2]
                if i == 0:
                    tensor.wait_ge(s_in, 32)
                if i >= 2:
                    j = i - 2
                    eng, cnt = cons_count[j]
                    tensor.wait_ge(s_cact if eng == "act" else s_cdve, cnt)
                for j in range(4):
                    mm = nc.tensor.matmul(
                        chunk[:, j * SUPER : (j + 1) * SUPER],
                        lhsT=u_sb[
                            :, R * SUPER + j * ROWT : R * SUPER + (j + 1) * ROWT
                        ],
                        rhs=v_sb[:, C * SUPER : (C + 1) * SUPER],
                        start=True,
                        stop=True,
                    )
                    if j == 3:
                        mm.then_inc(s_prod, 1)

        @block.scalar
        def _(scalar):
            for i in range(NSLOTS):
                if ENGINE[i] != "act":
                    continue
                chunk = chunks[i % 2]
                scalar.wait_ge(s_prod, i + 1)
                nc.scalar.activation(
                    out=act_dummy.ap()[:, i : i + 1].broadcast_to((128, CHUNK_FD)),
                    in_=chunk[:, :],
                    func=mybir.ActivationFunctionType.Sign,
                    accum_out=act_slots[:, SLOT_IDX[i] : SLOT_IDX[i] + 1],
                ).then_inc(s_cact, 1)

        @block.vector
        def _(vector):
            for i in range(NSLOTS):
                if ENGINE[i] != "dve":
                    continue
                chunk = chunks[i % 2]
                vector.wait_ge(s_prod, i + 1)
                nc.vector.tensor_scalar(
                    out=dve_dummy.ap()[:, i : i + 1].broadcast_to((128, CHUNK_FD)),
                    in0=chunk[:, :],
                    scalar1=0.0,
                    scalar2=None,
                    op0=mybir.AluOpType.is_gt,
                    op1=mybir.AluOpType.add,
                    accum_out=dve_slots[:, SLOT_IDX[i] : SLOT_IDX[i] + 1],
                ).then_inc(s_cdve, 1)

    return nc


def _prep_inputs(coords, atom_types, vdw_radii):
    """Host-side shard prep: per-batch u6/v6 [6, N] f32 arrays."""
    coords = np.asarray(coords, dtype=np.float32)  # [B, N, 3]
    atom_types = np.asarray(atom_types).astype(np.int64)  # [B, N]
    vdw_radii = np.asarray(vdw_radii, dtype=np.float32)  # [T]
    r = vdw_radii[atom_types]  # [B, N] f32 gather
    sq = np.einsum("bnd,bnd->bn", coords, coords, dtype=np.float32).astype(np.float32)
    s = (sq - r * r).astype(np.float32)
    in_maps = []
    for b in range(B):
        u = np.empty((K, N), np.float32)
        v = np.empty((K, N), np.float32)
        u[0:3] = coords[b].T
        v[0:3] = coords[b].T
        u[3] = r[b]
        v[3] = r[b]
        u[4] = -0.5 * s[b]
        v[4] = 1.0
        u[5] = 1.0
        v[5] = -0.5 * s[b]
        in_maps.append({"u6": u, "v6": v})
    return in_maps


def _combine(results):
    """Host-side gather: per-core count slots -> scalar loss."""
    chunk_elems = 128 * CHUNK_FD
    total = 0.0
    for b in range(B):
        counts = np.asarray(results[b]["counts"], np.float64)
        act = counts[:, :N_ACT].sum(axis=0)
        dve = counts[:, N_ACT:].sum(axis=0)
        cnt_b = 0.0
        for i, (R, C, w) in enumerate(SCHEDULE):
            if ENGINE[i] == "act":
                cnt = (chunk_elems + act[SLOT_IDX[i]]) / 2.0  # positives from sign-sum
            else:
                cnt = dve[SLOT_IDX[i]]
            cnt_b += w * cnt
        cnt_b -= N  # remove diagonal (G[n,n] = 2 r^2 > 0 always)
        total += (cnt_b / 2.0) / N
    return np.float32(total / B)


def kernel(coords, atom_types, vdw_radii):
    import sys

    if "/opt/trn_rl_repo" not in sys.path:
        sys.path.insert(0, "/opt/trn_rl_repo")
    from concourse.bass_utils import run_bass_kernel_spmd

    if "nc" not in _CACHE:
        _CACHE["nc"] = _build()
    nc = _CACHE["nc"]

    in_maps = _prep_inputs(coords, atom_types, vdw_radii)
    res = run_bass_kernel_spmd(nc, in_maps, core_ids=list(range(B)))
    return _combine(res.results)


if __name__ == "__main__":
    import sys

    sys.path.insert(0, "/root/problem")
    import reference as ref

    inputs = ref.setup_inputs()
    out = kernel(**{k: np.asarray(v) for k, v in inputs.items()})
    print("kernel output:", out)


# revision 8
# speedup vs baseline: 22.1983x; 22.1983x over previous
"""ClashLoss kernel for Trainium2 (8 NeuronCores, batch-parallel).

Math: for each batch b, count pairs (n, m), n != m, with
    dist(n, m) < radii[n] + radii[m]   and   dist(n, m) > EPS.
Using s_n = |c_n|^2 - r_n^2, the clash condition dist^2 < (r_n + r_m)^2 is
    G[n, m] = dot(c_n, c_m) + r_n r_m - s_n/2 - s_m/2 > 0,
i.e. a 6-dim dot product u_n . v_m with
    u_n = (x, y, z, r_n, -s_n/2, 1)     (matmul stationary side)
    v_m = (x, y, z, r_m, 1, -s_m/2)     (matmul moving side)
The PE computes G tile-by-tile into PSUM; ACT (Sign + accumulate) and DVE
(tensor_scalar is_gt + accumulate) count positives per 512x512 super-block.
Symmetry: only upper-triangular super-blocks are computed (weight 2), the
diagonal super-blocks once (weight 1).  The diagonal n == m has
G[n,n] = 2 r_n^2 > 0 always, so exactly N diagonal hits are subtracted on
the host.

Raw-bass implementation (not Tile): fp32 matmuls only support a single
sync-wait in codegen, so semaphores are placed by hand -- at most one wait
per matmul, standalone wait instructions elsewhere.
"""

import numpy as np

N = 4096
B = 8
K = 6  # augmented dot-product length
SUPER = 512  # super-block edge (4 PSUM banks wide)
NSUP = N // SUPER  # 8
ROWT = 128  # rows per matmul (output partitions)
CHUNK_FD = SUPER * 4  # 2048 f32 = 4 PSUM banks
EPS = 1e-8

# super-block schedule: (R, C, weight); R <= C
SCHEDULE = [(r, c, 1 if r == c else 2) for r in range(NSUP) for c in range(r, NSUP)]
NSLOTS = len(SCHEDULE)  # 36

# engine assignment: ACT is a bit faster per chunk; give it more supers.
ENGINE = ["act" if (i % 9) < 5 else "dve" for i in range(NSLOTS)]
# compact per-engine slot index for each super
SLOT_IDX = []
_na = _nd = 0
for _i in range(NSLOTS):
    if ENGINE[_i] == "act":
        SLOT_IDX.append(_na)
        _na += 1
    else:
        SLOT_IDX.append(_nd)
        _nd += 1
N_ACT, N_DVE = _na, _nd

_CACHE = {}


def _build(repeat=1):
    """Build the raw-bass SPMD program (same program for all cores).

    repeat > 1 re-runs the whole super-block schedule (for differential
    wall-clock timing); the counts are simply overwritten each pass.
    """
    import concourse.bass as bass
    from concourse import mybir

    nc = bass.Bass("TRN2", target_bir_lowering=False, debug=False)
    f32 = mybir.dt.float32

    u_dram = nc.dram_tensor("u6", [K, N], f32, kind="ExternalInput").ap()
    v_dram = nc.dram_tensor("v6", [K, N], f32, kind="ExternalInput").ap()
    out_dram = nc.dram_tensor(
        "counts", [128, N_ACT + N_DVE], f32, kind="ExternalOutput"
    ).ap()

    # flattened schedule over repeats
    gsched = SCHEDULE * repeat
    gengine = ENGINE * repeat
    gslot = SLOT_IDX * repeat
    ntot = len(gsched)

    # consumer bookkeeping: for global super g, which engine consumes it and
    # the cumulative per-engine consumption count up to and including g.
    cons_count = []  # (engine, count_after_g)
    na = nd = 0
    for g in range(ntot):
        if gengine[g] == "act":
            na += 1
            cons_count.append(("act", na))
        else:
            nd += 1
            cons_count.append(("dve", nd))
    n_act_total, n_dve_total = na, nd

    with (
        nc.sbuf_tensor([K, N], f32) as u_sb,
        nc.sbuf_tensor([K, N], f32) as v_sb,
        nc.sbuf_tensor([128, max(1, N_ACT)], f32) as act_slots,
        nc.sbuf_tensor([128, max(1, N_DVE)], f32) as dve_slots,
        nc.sbuf_tensor([128, NSLOTS], f32) as act_dummy,
        nc.sbuf_tensor([128, NSLOTS], f32) as dve_dummy,
        nc.psum_tensor([128, CHUNK_FD], f32) as chunk0,
        nc.psum_tensor([128, CHUNK_FD], f32) as chunk1,
        nc.semaphore("DMA_IN") as s_in,
        nc.semaphore("PROD") as s_prod,
        nc.semaphore("CACT") as s_cact,
        nc.semaphore("CDVE") as s_cdve,
        nc.semaphore("DMA_OUT") as s_out,
        nc.Block() as block,
    ):
        chunks = [chunk0, chunk1]

        @block.sync
        def _(sync):
            sync.dma_start(out=u_sb[:, :], in_=u_dram).then_inc(s_in, 16)
            sync.dma_start(out=v_sb[:, :], in_=v_dram).then_inc(s_in, 16)
            sync.wait_ge(s_cact, n_act_total)
            sync.wait_ge(s_cdve, n_dve_total)
            sync.dma_start(
                out=out_dram[:, 0:N_ACT], in_=act_slots[:, :]
            ).then_inc(s_out, 16)
            sync.dma_start(
                out=out_dram[:, N_ACT : N_ACT + N_DVE], in_=dve_slots[:, :]
            ).then_inc(s_out, 16)
            sync.wait_ge(s_out, 32)

        @block.tensor
        def _(tensor):
            for g in range(ntot):
                R, C, _w = gsched[g]
                chunk = chunks[g % 2]
                if g == 0:
                    tensor.wait_ge(s_in, 32)
                if g >= 2:
                    eng, cnt = cons_count[g - 2]
                    tensor.wait_ge(s_cact if eng == "act" else s_cdve, cnt)
                for j in range(4):
                    mm = nc.tensor.matmul(
                        chunk[:, j * SUPER : (j + 1) * SUPER],
                        lhsT=u_sb[
                            :, R * SUPER + j * ROWT : R * SUPER + (j + 1) * ROWT
                        ],
                        rhs=v_sb[:, C * SUPER : (C + 1) * SUPER],
                        start=True,
                        stop=True,
                    )
                    if j == 3:
                        mm.then_inc(s_prod, 1)

        @block.scalar
        def _(scalar):
            for g in range(ntot):
                if gengine[g] != "act":
                    continue
                i = g % NSLOTS
                chunk = chunks[g % 2]
                scalar.wait_ge(s_prod, g + 1)
                nc.scalar.activation(
                    out=act_dummy.ap()[:, i : i + 1].broadcast_to((128, CHUNK_FD)),
                    in_=chunk[:, :],
                    func=mybir.ActivationFunctionType.Sign,
                    accum_out=act_slots[:, gslot[g] : gslot[g] + 1],
                ).then_inc(s_cact, 1)

        @block.vector
        def _(vector):
            for g in range(ntot):
                if gengine[g] != "dve":
                    continue
                i = g % NSLOTS
                chunk = chunks[g % 2]
                vector.wait_ge(s_prod, g + 1)
                nc.vector.tensor_scalar(
                    out=dve_dummy.ap()[:, i : i + 1].broadcast_to((128, CHUNK_FD)),
                    in0=chunk[:, :],
                    scalar1=0.0,
                    scalar2=None,
                    op0=mybir.AluOpType.is_gt,
                    op1=mybir.AluOpType.add,
                    accum_out=dve_slots[:, gslot[g] : gslot[g] + 1],
                ).then_inc(s_cdve, 1)

    return nc


def _prep_inputs(coords, atom_types, vdw_radii):
    """Host-side shard prep: per-batch u6/v6 [6, N] f32 arrays."""
    coords = np.asarray(coords, dtype=np.float32)  # [B, N, 3]
    atom_types = np.asarray(atom_types).astype(np.int64)  # [B, N]
    vdw_radii = np.asarray(vdw_radii, dtype=np.float32)  # [T]
    r = vdw_radii[atom_types]  # [B, N] f32 gather
    sq = np.einsum("bnd,bnd->bn", coords, coords, dtype=np.float32).astype(np.float32)
    s = (sq - r * r).astype(np.float32)
    in_maps = []
    for b in range(B):
        u = np.empty((K, N), np.float32)
        v = np.empty((K, N), np.float32)
        u[0:3] = coords[b].T
        v[0:3] = coords[b].T
        u[3] = r[b]
        v[3] = r[b]
        u[4] = -0.5 * s[b]
        v[4] = 1.0
        u[5] = 1.0
        v[5] = -0.5 * s[b]
        in_maps.append({"u6": u, "v6": v})
    return in_maps


def _combine(results):
    """Host-side gather: per-core count slots -> scalar loss."""
    chunk_elems = 128 * CHUNK_FD
    total = 0.0
    for b in range(B):
        counts = np.asarray(results[b]["counts"], np.float64)
        act = counts[:, :N_ACT].sum(axis=0)
        dve = counts[:, N_ACT:].sum(axis=0)
        cnt_b = 0.0
        for i, (R, C, w) in enumerate(SCHEDULE):
            if ENGINE[i] == "act":
                cnt = (chunk_elems + act[SLOT_IDX[i]]) / 2.0  # positives from sign-sum
            else:
                cnt = dve[SLOT_IDX[i]]
            cnt_b += w * cnt
        cnt_b -= N  # remove diagonal (G[n,n] = 2 r^2 > 0 always)
        total += (cnt_b / 2.0) / N
    return np.float32(total / B)


def kernel(coords, atom_types, vdw_radii):
    import sys

    if "/opt/trn_rl_repo" not in sys.path:
        sys.path.insert(0, "/opt/trn_rl_repo")
    from concourse.bass_utils import run_bass_kernel_spmd

    if "nc" not in _CACHE:
        _CACHE["nc"] = _build()
    nc = _CACHE["nc"]

    in_maps = _prep_inputs(coords, atom_types, vdw_radii)
    res = run_bass_kernel_spmd(nc, in_maps, core_ids=list(range(B)))
    return _combine(res.results)


if __name__ == "__main__":
    import sys

    sys.path.insert(0, "/root/problem")
    import reference as ref

    inputs = ref.setup_inputs()
    out = kernel(**{k: np.asarray(v) for k, v in inputs.items()})
    print("kernel output:", out)
